# revision 1
# baseline (speedup 1.0000x reference)
"""Multi-head attention Trainium2 kernel (8 NeuronCores, data-parallel over batch).

Per-core program (2 batches per core):
  x [2048, 512] (row-major [t, c] per batch)
  -> PE-transpose to xT [c, t] (f32)
  -> QKV projections in float32r (FP22, full-rate): qT/kT [d, t] per head-pair,
     V [t, hd] (stored bf16)
  -> scores S^T [s, tq] per (pair, s-tile, head), K=64 row-tiled so the two
     heads of a pair run concurrently in the PE array (f32r)
  -> exp on ScalarE (scale=1/8 folded in), PSUM -> SBUF bf16
  -> PV + rowsum in bf16, column-tiled: O pair occupies PSUM partitions
     0:64 / 64:128, rowsum broadcast comes from an all-ones stationary
  -> normalize on VectorE (reciprocal + multiply) -> O^T [hd, t] f32r
  -> output projection f32r + bias add -> y [2048, 512]
"""
import sys
import os

sys.path.insert(0, "/opt/trn_rl_repo")
import numpy as np

B, C, HH, WW = 16, 512, 32, 32
T = HH * WW              # 1024
NH, HD = 8, 64
BL = 2                   # batches per core
NCORES = 8

_CACHE = {}


def _build_nc():
    import concourse.bacc as bacc
    import concourse.mybir as mybir
    import concourse.tile as tile
    from concourse import masks

    f32 = mybir.dt.float32
    f32r = mybir.dt.float32r
    bf16 = mybir.dt.bfloat16
    Exp = mybir.ActivationFunctionType.Exp

    nc = bacc.Bacc("TRN2", target_bir_lowering=False, debug=False, num_devices=NCORES)
    x = nc.dram_tensor("x", [BL * T, C], f32, kind="ExternalInput").ap()
    wq = nc.dram_tensor("wq", [128, 2048], f32, kind="ExternalInput").ap()
    wk = nc.dram_tensor("wk", [128, 2048], f32, kind="ExternalInput").ap()
    wv = nc.dram_tensor("wv", [128, 2048], f32, kind="ExternalInput").ap()
    wp = nc.dram_tensor("wp", [128, 2048], f32, kind="ExternalInput").ap()
    bp = nc.dram_tensor("bp", [1, C], f32, kind="ExternalInput").ap()
    y = nc.dram_tensor("y", [BL * T, C], f32, kind="ExternalOutput").ap()

    with tile.TileContext(nc) as tc:
        with tc.tile_pool(name="const", bufs=1) as cpool, \
             tc.tile_pool(name="xnat", bufs=3) as xn_pool, \
             tc.tile_pool(name="xt", bufs=1) as xt_pool, \
             tc.tile_pool(name="qk", bufs=8) as qk_pool, \
             tc.tile_pool(name="vv", bufs=16) as v_pool, \
             tc.tile_pool(name="pp", bufs=28) as p_pool, \
             tc.tile_pool(name="ot", bufs=5) as ot_pool, \
             tc.tile_pool(name="rc", bufs=2) as rc_pool, \
             tc.tile_pool(name="yy", bufs=3) as y_pool, \
             tc.tile_pool(name="ps", bufs=4, space="PSUM") as ps_pool:

            wq_s = cpool.tile([128, 2048], f32r, tag="wq")
            wk_s = cpool.tile([128, 2048], f32r, tag="wk")
            wv_s = cpool.tile([128, 2048], f32r, tag="wv")
            wp_s = cpool.tile([128, 2048], f32r, tag="wp")
            nc.sync.dma_start(wq_s[:], wq.bitcast(f32r))
            nc.sync.dma_start(wk_s[:], wk.bitcast(f32r))
            nc.sync.dma_start(wv_s[:], wv.bitcast(f32r))
            nc.sync.dma_start(wp_s[:], wp.bitcast(f32r))
            bias_b = cpool.tile([128, C], f32, tag="bias")
            nc.sync.dma_start(bias_b[:], bp.to_broadcast([128, C]))
            ones_bf = cpool.tile([128, HD], bf16, tag="ones")
            nc.gpsimd.memset(ones_bf[:], 1.0)
            ident = cpool.tile([128, 128], f32, tag="ident")
            masks.make_identity(nc, ident[:])

            def prep(b):
                # ---- load + transpose x -> xT [c_local, cc, t] ----
                xts = xt_pool.tile([128, 4, T], f32r, tag="xt", name=f"xts_{b}")
                for tt in range(8):
                    xn = xn_pool.tile([128, C], f32, tag="xn", name=f"xn_{b}_{tt}")
                    nc.sync.dma_start(xn[:], x[b * T + tt * 128: b * T + tt * 128 + 128, :])
                    tr = ps_pool.tile([128, C], f32, tag="ps", name=f"tr_{b}_{tt}")
                    for cc in range(4):
                        nc.tensor.transpose(tr[:, cc * 128:(cc + 1) * 128],
                                            xn[:, cc * 128:(cc + 1) * 128], ident[:])
                    nc.vector.tensor_copy(xts[:, :, tt * 128:(tt + 1) * 128],
                                          tr[:].rearrange("p (cc m) -> p cc m", cc=4))

                # ---- QKV projections ----
                qts, kts = [], []
                for p in range(4):
                    for wi, (wsb, lst) in enumerate(((wq_s, qts), (wk_s, kts))):
                        ps_t = ps_pool.tile([128, T], f32, tag="ps", name=f"qk_{b}_{p}_{wi}")
                        for ch in range(2):
                            for cc in range(4):
                                nc.tensor.matmul(
                                    ps_t[:, ch * 512:(ch + 1) * 512],
                                    wsb[:, cc * 512 + p * 128: cc * 512 + p * 128 + 128],
                                    xts[:, cc, ch * 512:(ch + 1) * 512],
                                    start=(cc == 0), stop=(cc == 3))
                        sb_t = qk_pool.tile([128, T], f32r, tag="qk", name=f"qks_{b}_{p}_{wi}")
                        nc.vector.tensor_copy(sb_t[:], ps_t[:])
                        lst.append(sb_t)
                vts = []
                for st in range(8):
                    ps_t = ps_pool.tile([128, C], f32, tag="ps", name=f"v_{b}_{st}")
                    for cc in range(4):
                        nc.tensor.matmul(ps_t[:],
                                         xts[:, cc, st * 128:(st + 1) * 128],
                                         wv_s[:, cc * 512:(cc + 1) * 512],
                                         start=(cc == 0), stop=(cc == 3))
                    v_t = v_pool.tile([128, C], bf16, tag="v", name=f"vs_{b}_{st}")
                    nc.vector.tensor_copy(v_t[:], ps_t[:])
                    vts.append(v_t)
                return qts, kts, vts

            def attention(b, qts, kts, vts):
                # ---- attention, one head-pair at a time ----
                # Phase 1 per pair: all scores + exp (P~ for the whole pair
                # lives in SBUF).  Phase 2: PV+rowsum in two tq halves so
                # o/r only pin one PSUM bank each, leaving slots for the
                # next pair's scores/exp (and next batch's QKV) to overlap.
                ots = []
                for p in range(4):
                    pjs = {}
                    for j in range(8):
                        s_list = [ps_pool.tile([128, T], f32, tag="ps", name=f"s_{b}_{p}_{j}_{h}")
                                  for h in range(2)]
                        for ch in range(2):
                            for h in range(2):
                                nc.tensor.matmul(
                                    s_list[h][:, ch * 512:(ch + 1) * 512],
                                    kts[p][h * 64:h * 64 + 64, j * 128:(j + 1) * 128],
                                    qts[p][h * 64:h * 64 + 64, ch * 512:(ch + 1) * 512])
                        for h in range(2):
                            p_sb = p_pool.tile([128, T], bf16, tag="p", name=f"p_{b}_{p}_{j}_{h}")
                            nc.scalar.activation(p_sb[:], s_list[h][:], Exp, scale=0.125)
                            pjs[(j, h)] = p_sb
                    ot = ot_pool.tile([128, T], f32r, tag="ot", name=f"ot_{b}_{p}")
                    for tq in range(2):
                        # O pair in bank 0 (cols 0:512), rowsum pair in bank 1
                        # (cols 512:1024): one PSUM slot per tq half, so the
                        # next half's matmuls need not wait for this half's
                        # DVE normalize to release two slots.
                        or_ps = ps_pool.tile([128, 1024], f32, tag="ps", name=f"or_{b}_{p}_{tq}")
                        for j in range(8):
                            for h in range(2):
                                nc.tensor.matmul(
                                    or_ps[h * 64:h * 64 + 64, 0:512],
                                    vts[j][:, (2 * p + h) * 64:(2 * p + h) * 64 + 64],
                                    pjs[(j, h)][:, tq * 512:(tq + 1) * 512],
                                    start=(j == 0), stop=(j == 7),
                                    skip_group_check=True)
                            for h in range(2):
                                nc.tensor.matmul(
                                    or_ps[h * 64:h * 64 + 64, 512:1024],
                                    ones_bf[:, 0:HD],
                                    pjs[(j, h)][:, tq * 512:(tq + 1) * 512],
                                    start=(j == 0), stop=(j == 7),
                                    skip_group_check=True)
                        rec = rc_pool.tile([128, 512], f32, tag="rc", name=f"rec_{b}_{p}_{tq}")
                        nc.vector.reciprocal(rec[:], or_ps[:, 512:1024])
                        nc.vector.tensor_mul(ot[:, tq * 512:(tq + 1) * 512], or_ps[:, 0:512], rec[:])
                    ots.append(ot)
                return ots

            def proj(b, ots):
                # ---- output projection + bias ----
                for tt in range(8):
                    y_ps = ps_pool.tile([128, C], f32, tag="ps", name=f"y_{b}_{tt}")
                    for p in range(4):
                        nc.tensor.matmul(y_ps[:],
                                         ots[p][:, tt * 128:(tt + 1) * 128],
                                         wp_s[:, p * 512:(p + 1) * 512],
                                         start=(p == 0), stop=(p == 3))
                    y_sb = y_pool.tile([128, C], f32, tag="y", name=f"ys_{b}_{tt}")
                    nc.vector.tensor_add(y_sb[:], y_ps[:], bias_b[:])
                    nc.sync.dma_start(y[b * T + tt * 128: b * T + tt * 128 + 128, :], y_sb[:])

            # Emission order: hoist batch 1's load/transpose/QKV before
            # batch 0's projection so the scheduler can fill batch 0's
            # exp-gated attention windows with batch 1 PE work.
            q0 = prep(0)
            ot0 = attention(0, *q0)
            q1 = prep(1)
            proj(0, ot0)
            ot1 = attention(1, *q1)
            proj(1, ot1)

    nc.compile()
    return nc


def _pack_qk(w):
    # [NH, C, HD] -> [c, h*HD+d] -> tiled [c_local, cc, p, m] -> [128, 2048]
    wn = np.transpose(w, (1, 0, 2)).reshape(C, C)
    return np.ascontiguousarray(
        wn.reshape(4, 128, 4, 128).transpose(1, 0, 2, 3).reshape(128, 2048))


def _pack_cn(wn):
    # [C, N] natural -> tiled [c_local, cc, n] -> [128, 2048]
    return np.ascontiguousarray(wn.reshape(4, 128, C).transpose(1, 0, 2).reshape(128, 2048))


def get_nc():
    if "nc" not in _CACHE:
        _CACHE["nc"] = _build_nc()
    return _CACHE["nc"]


def make_in_maps(x, Wq, Wk, Wv, Wproj, bproj):
    x = np.asarray(x, dtype=np.float32)
    wq_t = _pack_qk(np.asarray(Wq, np.float32))
    wk_t = _pack_qk(np.asarray(Wk, np.float32))
    wv_t = _pack_cn(np.transpose(np.asarray(Wv, np.float32), (1, 0, 2)).reshape(C, C))
    wp_t = _pack_cn(np.asarray(Wproj, np.float32))
    bp_t = np.asarray(bproj, np.float32).reshape(1, C)
    in_maps = []
    for i in range(NCORES):
        in_maps.append({
            "x": np.ascontiguousarray(x[BL * i: BL * (i + 1)].reshape(BL * T, C)),
            "wq": wq_t, "wk": wk_t, "wv": wv_t, "wp": wp_t, "bp": bp_t,
        })
    return in_maps


def kernel(x, Wq, Wk, Wv, Wproj, bproj):
    from concourse.bass_utils import run_bass_kernel_spmd

    nc = get_nc()
    in_maps = make_in_maps(x, Wq, Wk, Wv, Wproj, bproj)
    trace = bool(int(os.environ.get("KERNEL_TRACE", "0")))
    res = run_bass_kernel_spmd(nc, in_maps, list(range(NCORES)), trace=trace)
    _CACHE["last_result"] = res
    out = np.empty((B, C, HH, WW), np.float32)
    for i in range(NCORES):
        out[BL * i: BL * (i + 1)] = res.results[i]["y"].reshape(BL, C, HH, WW)
    return out



# revision 47
# speedup vs baseline: 1.9150x; 1.9150x over previous
"""Multi-head attention Trainium2 kernel (8 NeuronCores, data-parallel over batch).

Per-core program (2 batches per core), cost-model-optimized, all-bf16:
  x [2048, 512] bf16 --DMA-xbar-transpose--> xts [c,(cc),t] bf16
  QKV projections bf16 (c-tiled accumulation, N=512 per step)
  scores S^T[s,t] = kT (stationary) x qT (moving) bf16 -> exp on ScalarE
  (optional DVE Schraudolph offload) -> P [s,t] bf16
  PV in O[t,d] layout: P^T stationary, V_ext (V|ones) moving bf16, N=65 per
  accumulation step -> rowsum free in col 64; two heads per psum tile in
  disjoint banks (start=True zeroes a whole 2KB bank)
  normalize on GPSIMD (normalize_recip, SBUF->SBUF) -> O_sb bf16
  O_sb -> PE-transpose -> oT [hd,t] bf16 -> proj bf16 + bias add -> y f32
Emission is interleaved in head-pair windows so PE/DVE/Pool work hides
under the ACT (exp) critical path.
"""
import sys
import os
from collections import deque

sys.path.insert(0, "/opt/trn_rl_repo")
import numpy as np
import ml_dtypes

B, C, HH, WW = 16, 512, 32, 32
T = HH * WW               # 1024
NH, HD = 8, 64
BL = 2                    # batches per core
NCORES = 8
EXP_SCALE = 0.125

FP8 = ml_dtypes.float8_e4m3
BF16 = ml_dtypes.bfloat16

# s-tile indices whose exp runs on DVE (Schraudolph bit-trick) instead of ACT
SCHR_ST = set(int(c) for c in os.environ.get("SCHR_ST", "").split(",") if c != "")
# Schraudolph constants for bf16 bits: bits = 2^7 * (log2(P) + 127 - c)
SCHR_A = 128.0 * np.log2(np.e) * EXP_SCALE
SCHR_B = 128.0 * (127.0 - 0.0579) + float(os.environ.get("SCHR_RND", "0.5"))

_CACHE = {}


def _build_nc():
    import concourse.bacc as bacc
    import concourse.mybir as mybir
    import concourse.tile as tile
    from concourse import masks
    from concourse import library_config

    f32 = mybir.dt.float32
    bf16 = mybir.dt.bfloat16
    u16 = mybir.dt.uint16
    Exp = mybir.ActivationFunctionType.Exp
    MUL = mybir.AluOpType.mult

    nc = bacc.Bacc("TRN2", target_bir_lowering=False, debug=False, num_devices=NCORES)
    x = nc.dram_tensor("x", [BL * T, C], bf16, kind="ExternalInput").ap()
    wq = nc.dram_tensor("wq", [128, 2048], bf16, kind="ExternalInput").ap()
    wk = nc.dram_tensor("wk", [128, 2048], bf16, kind="ExternalInput").ap()
    wv = nc.dram_tensor("wv", [128, 2048], bf16, kind="ExternalInput").ap()
    wp = nc.dram_tensor("wp", [128, 2048], bf16, kind="ExternalInput").ap()
    bp = nc.dram_tensor("bp", [1, C], f32, kind="ExternalInput").ap()
    y = nc.dram_tensor("y", [BL * T, C], f32, kind="ExternalOutput").ap()

    with tile.TileContext(nc) as tc:
        with tc.tile_pool(name="const", bufs=1) as cpool, \
             tc.tile_pool(name="xn", bufs=4) as xn_pool, \
             tc.tile_pool(name="xt", bufs=2) as xt_pool, \
             tc.tile_pool(name="qk", bufs=16) as qk_pool, \
             tc.tile_pool(name="vv", bufs=16) as v_pool, \
             tc.tile_pool(name="pp", bufs=34) as p_pool, \
             tc.tile_pool(name="ob", bufs=2) as osb_pool, \
             tc.tile_pool(name="ot", bufs=8) as ot_pool, \
             tc.tile_pool(name="or", bufs=3) as or_pool, \
             tc.tile_pool(name="yy", bufs=5) as y_pool, \
             tc.tile_pool(name="ps", bufs=1, space="PSUM") as ps_pool:

            wq_s = cpool.tile([128, 4, C], bf16, tag="wq")
            wk_s = cpool.tile([128, 4, C], bf16, tag="wk")
            wv_s = cpool.tile([128, 4, C], bf16, tag="wv")
            wp_s = cpool.tile([128, 2048], bf16, tag="wp")
            bias_b = cpool.tile([128, C], f32, tag="bias")
            ident = cpool.tile([128, 128], bf16, tag="ident")
            # DMA order matters: the batch-0 x tiles and wq/wk gate the first
            # scores, so everything else loads after those.
            masks.make_identity(nc, ident[:])
            nc.gpsimd.load_library(library_config.attn)

            def load_wq():
                nc.sync.dma_start(wq_s[:], wq.rearrange("p (a b) -> p a b", a=4))

            def load_wk():
                nc.sync.dma_start(wk_s[:], wk.rearrange("p (a b) -> p a b", a=4))

            def load_consts_rest():
                nc.sync.dma_start(wv_s[:], wv.rearrange("p (a b) -> p a b", a=4))
                nc.sync.dma_start(wp_s[:], wp)
                nc.sync.dma_start(bias_b[:], bp.to_broadcast([128, C]))

            # persistent per-batch state
            xts = {}      # b -> [128, 4, 1024] bf16
            qsb, ksb = {}, {}   # (b, pair) -> [128, 1024] bf16 (qT/kT, 2 heads)
            vsb = {}      # (b, j) -> [128, 8, 65] bf16 (V | ones)
            Pt = {}       # (b, h, st) -> [128, 1024] bf16
            Osb = {}      # b -> [128, 4096] bf16
            oT = {}       # (b, pair) -> [128, 1024] bf16

            def u_tile(name):
                return ps_pool.tile([128, 512], f32, tag="u", bufs=4, name=name)

            def s_tile(name):
                return ps_pool.tile([128, 1024], f32, tag="s", bufs=2, name=name)

            # ---------- prep units ----------
            def prep_units(b, units):
                xts[b] = xt_pool.tile([128, 4, T], bf16, tag="xt", name=f"xts_{b}")
                Osb[b] = osb_pool.tile([128, 8 * C], bf16, tag="osb", name=f"osb_{b}")

                def xt_unit(cc):
                    # batch 1: xbar DMA transpose (no PE/DVE cost, runs in
                    # window slack)
                    def emit():
                        nc.sync.dma_start_transpose(
                            xts[b][:, cc, :],
                            x[b * T:(b + 1) * T, cc * 128:(cc + 1) * 128])
                    return emit

                def xn_unit(tt):
                    # batch 0: PE transpose route — much shorter critical
                    # path at kernel start than 4 serial xbar transposes
                    def emit():
                        xn = xn_pool.tile([128, C], bf16, tag="xn", name=f"xn_{b}_{tt}")
                        nc.sync.dma_start(xn[:], x[b * T + tt * 128: b * T + tt * 128 + 128, :])
                        tr = u_tile(f"tr_{b}_{tt}").bitcast(bf16)
                        for cc in range(4):
                            nc.tensor.transpose(tr[:, cc * 128:(cc + 1) * 128],
                                                xn[:, cc * 128:(cc + 1) * 128], ident[:])
                        nc.vector.tensor_copy(
                            xts[b][:, :, tt * 128:(tt + 1) * 128],
                            tr[:, 0:512].rearrange("p (cc m) -> p cc m", cc=4))
                    return emit

                def qk_unit(wi, p, ch):
                    def emit():
                        wsb = (wq_s, wk_s)[wi]
                        dst = (qsb, ksb)[wi]
                        if (b, p) not in dst:
                            dst[(b, p)] = qk_pool.tile(
                                [128, T], bf16, tag="qk", name=f"qk_{b}_{wi}_{p}")
                        ps_t = u_tile(f"qkp_{b}_{wi}_{p}_{ch}")
                        for cc in range(4):
                            nc.tensor.matmul(
                                ps_t[:],
                                wsb[:, cc, p * 128:(p + 1) * 128],
                                xts[b][:, cc, ch * 512:(ch + 1) * 512],
                                start=(cc == 0), stop=(cc == 3))
                        nc.vector.tensor_copy(
                            dst[(b, p)][:, ch * 512:(ch + 1) * 512], ps_t[:])
                    return emit

                def v_unit(st):
                    def emit():
                        v_ps = u_tile(f"vp_{b}_{st}")
                        for cc in range(4):
                            nc.tensor.matmul(
                                v_ps[:],
                                xts[b][:, cc, st * 128:(st + 1) * 128],
                                wv_s[:, cc, :],
                                start=(cc == 0), stop=(cc == 3))
                        v_t = v_pool.tile([128, NH, HD + 1], bf16, tag="v",
                                          name=f"vs_{b}_{st}")
                        nc.gpsimd.memset(v_t[:, :, HD], 1.0)
                        nc.vector.tensor_copy(
                            v_t[:, :, 0:HD],
                            v_ps[:].rearrange("p (h d) -> p h d", h=NH))
                        vsb[(b, st)] = v_t
                    return emit

                if b == 0:
                    units["xt", b] = [xn_unit(tt) for tt in range(8)]
                else:
                    units["xt", b] = [xt_unit(cc) for cc in range(4)]
                for p in range(4):
                    units["qk", b, p] = [qk_unit(wi, p, ch)
                                         for wi in range(2) for ch in range(2)]
                units["v", b] = [v_unit(st) for st in range(8)]

            # ---------- attention pieces ----------
            def scores_exp(b, h, st, split=False):
                s_ps = s_tile(f"s_{b}_{h}_{st}")
                p, hh = h // 2, h % 2
                p_t = p_pool.tile([128, T], bf16, tag="p", name=f"p_{b}_{h}_{st}")
                for ch in range(2):
                    nc.tensor.matmul(
                        s_ps[:, ch * 512:(ch + 1) * 512],
                        ksb[(b, p)][64 * hh:64 * hh + 64, st * 128:(st + 1) * 128],
                        qsb[(b, p)][64 * hh:64 * hh + 64, ch * 512:(ch + 1) * 512],
                        start=True, stop=True)
                    if split:
                        # half-width exp right after each scores half: lets
                        # ACT start before the second half's operands exist
                        nc.scalar.activation(
                            p_t[:, ch * 512:(ch + 1) * 512],
                            s_ps[:, ch * 512:(ch + 1) * 512], Exp, scale=EXP_SCALE)
                if not split:
                    if st in SCHR_ST:
                        nc.vector.tensor_scalar(
                            p_t[:].bitcast(u16), s_ps[:],
                            scalar1=float(SCHR_A), op0=MUL,
                            scalar2=float(SCHR_B), op1=mybir.AluOpType.add)
                    else:
                        nc.scalar.activation(p_t[:], s_ps[:], Exp, scale=EXP_SCALE)
                Pt[(b, h, st)] = p_t

            def pv_chunk(b, pr, tt):
                # one psum bank (u tile) per head: each accumulation group
                # owns its bank (start=True zeroes a whole 2KB bank).
                ha = 2 * pr
                o_ps = [u_tile(f"o_{b}_{pr}_{tt}_{hi}") for hi in range(2)]
                for j in range(8):
                    for hi in range(2):
                        nc.tensor.matmul(
                            o_ps[hi][:, 0:HD + 1],
                            Pt[(b, ha + hi, j)][:, tt * 128:(tt + 1) * 128],
                            vsb[(b, j)][:, ha + hi, :],
                            start=(j == 0), stop=(j == 7),
                            skip_group_check=True)
                o_raw = or_pool.tile([128, 2, HD + 1], f32, tag="or",
                                     name=f"oraw_{b}_{pr}_{tt}")
                for hi in range(2):
                    nc.vector.tensor_copy(o_raw[:, hi, :], o_ps[hi][:, 0:HD + 1])
                for hi in range(2):
                    nc.gpsimd.normalize_recip(
                        Osb[b][:, tt * 512 + (ha + hi) * HD:
                               tt * 512 + (ha + hi + 1) * HD],
                        o_raw[:, hi, 0:HD],
                        o_raw[:, hi, HD:HD + 1])

            def ot_unit(b, pr):
                otp = u_tile(f"otp_{b}_{pr}").bitcast(bf16)
                for tt in range(8):
                    nc.tensor.transpose(
                        otp[:, tt * 128:(tt + 1) * 128],
                        Osb[b][:, tt * 512 + pr * 128: tt * 512 + (pr + 1) * 128],
                        ident[:])
                o_t = ot_pool.tile([128, T], bf16, tag="ot", name=f"oT_{b}_{pr}")
                nc.vector.tensor_copy(o_t[:], otp[:])
                oT[(b, pr)] = o_t

            y01 = {}

            def proj_units(b):
                units = []

                def proj_unit(tt):
                    def emit():
                        y_ps = u_tile(f"yp_{b}_{tt}")
                        for p in range(4):
                            nc.tensor.matmul(y_ps[:],
                                             oT[(b, p)][:, tt * 128:(tt + 1) * 128],
                                             wp_s[:, p * 512:(p + 1) * 512],
                                             start=(p == 0), stop=(p == 3))
                        y_sb = y_pool.tile([128, C], f32, tag="y", name=f"ys_{b}_{tt}")
                        nc.vector.tensor_add(y_sb[:], y_ps[:], bias_b[:])
                        nc.sync.dma_start(y[b * T + tt * 128: b * T + tt * 128 + 128, :],
                                          y_sb[:])
                    return emit

                for tt in range(8):
                    units.append(proj_unit(tt))
                return units

            def proj01_units(b):
                # first half of the output projection (pairs 0/1 + bias),
                # runnable as soon as oT[(b,0..1)] exist — fills window slack
                units = []

                def unit(tt):
                    def emit():
                        y_ps = u_tile(f"yh_{b}_{tt}")
                        for p in range(2):
                            nc.tensor.matmul(y_ps[:],
                                             oT[(b, p)][:, tt * 128:(tt + 1) * 128],
                                             wp_s[:, p * 512:(p + 1) * 512],
                                             start=(p == 0), stop=(p == 1))
                        yh = y_pool.tile([128, C], bf16, tag="y01", bufs=8,
                                         name=f"yh_{b}_{tt}")
                        nc.vector.tensor_add(yh[:], y_ps[:], bias_b[:])
                        y01[(b, tt)] = yh
                    return emit

                for tt in range(8):
                    units.append(unit(tt))
                return units

            def proj23_units(b):
                units = []

                def unit(tt):
                    def emit():
                        y_ps = u_tile(f"yt_{b}_{tt}")
                        for p in range(2, 4):
                            nc.tensor.matmul(y_ps[:],
                                             oT[(b, p)][:, tt * 128:(tt + 1) * 128],
                                             wp_s[:, p * 512:(p + 1) * 512],
                                             start=(p == 2), stop=(p == 3))
                        y_sb = y_pool.tile([128, C], f32, tag="y", name=f"ys_{b}_{tt}")
                        nc.vector.tensor_tensor(y_sb[:], y_ps[:], y01[(b, tt)][:],
                                                op=mybir.AluOpType.add)
                        nc.sync.dma_start(y[b * T + tt * 128: b * T + tt * 128 + 128, :],
                                          y_sb[:])
                    return emit

                for tt in range(8):
                    units.append(unit(tt))
                return units

            # ---------- emission: 8 head-pair windows ----------
            U = {}
            prep_units(0, U)
            prep_units(1, U)
            # head: x transposes + pair-0 q/k of batch 0. ch0 of q/k only
            # needs the first 4 t-tiles transposed, so interleave.
            qk00 = U["qk", 0, 0]   # order: (q,ch0), (q,ch1), (k,ch0), (k,ch1)
            for u in U["xt", 0][0:4]:
                u()
            load_wq()
            load_wk()
            qk00[0]()
            for u in U["xt", 0][4:6]:
                u()
            qk00[2]()
            for u in U["xt", 0][6:8]:
                u()
            qk00[1]()
            qk00[3]()
            load_consts_rest()
            # per-window filler schedule (deadline: qk(b,p) before window of
            # pair (b,p); v(b) before the first pv_chunk of batch b's pairs)
            wfill = [
                U["qk", 0, 1] + U["v", 0],          # W0 (no pv in W0)
                U["qk", 0, 2] + U["xt", 1],         # W1
                U["qk", 0, 3] + U["qk", 1, 0],      # W2
                U["qk", 1, 1] + U["qk", 1, 2],      # W3
                U["v", 1],                          # W4
                U["qk", 1, 3],                      # W5
                proj_units(0),                      # W6
                proj01_units(1),                    # W7
            ]
            pairs = [(b, pr) for b in range(2) for pr in range(4)]
            for wi_, (b, pr) in enumerate(pairs):
                # oT for the pair whose PV finished at the end of last window
                if wi_ >= 2:
                    ot_unit(*pairs[wi_ - 2])
                fill = deque(wfill[wi_])
                per_st = (len(fill) + 7) // 8
                for st in range(8):
                    for hi in range(2):
                        scores_exp(b, 2 * pr + hi, st,
                                   split=(wi_ == 0 and st < 2))
                    if wi_ >= 1:
                        pb, ppr = pairs[wi_ - 1]
                        pv_chunk(pb, ppr, st)
                    for _ in range(per_st):
                        if fill:
                            fill.popleft()()
                while fill:
                    fill.popleft()()
            # tail: fully pipelined per tt — pv -> transpose -> oT-slice copy
            # -> proj23 -> y add -> store (s pool is free for otp3 now)
            ot_unit(1, 2)
            otp3 = s_tile("otp_1_3").bitcast(bf16)
            o_t3 = ot_pool.tile([128, T], bf16, tag="ot", name="oT_1_3")
            oT[(1, 3)] = o_t3
            p23 = proj23_units(1)

            def tail_unit(tt):
                # transpose + oT-slice copy only; proj23 emitted after all
                # copies so the DVE queue never blocks the next tt's copy
                nc.tensor.transpose(
                    otp3[:, tt * 128:(tt + 1) * 128],
                    Osb[1][:, tt * 512 + 3 * 128: tt * 512 + 4 * 128],
                    ident[:])
                nc.vector.tensor_copy(o_t3[:, tt * 128:(tt + 1) * 128],
                                      otp3[:, tt * 128:(tt + 1) * 128])

            for tt in range(8):
                pv_chunk(1, 3, tt)
                if tt >= 3:
                    tail_unit(tt - 3)
            for tt in range(5, 8):
                tail_unit(tt)
            for tt in range(8):
                p23[tt]()

    nc.compile()
    return nc


def _pack_qk(w):
    # [NH, C, HD] -> [c, h*HD+d] -> tiled [c_local, cc, d] -> [128, 2048] bf16
    wn = np.transpose(np.asarray(w, np.float32), (1, 0, 2)).reshape(C, C)
    return np.ascontiguousarray(
        wn.reshape(4, 128, C).transpose(1, 0, 2).reshape(128, 2048)).astype(BF16)


def _pack_cn(wn):
    # [C, N] natural -> tiled [c_local, cc, n] -> [128, 2048] bf16
    return np.ascontiguousarray(
        np.asarray(wn, np.float32).reshape(4, 128, C)
        .transpose(1, 0, 2).reshape(128, 2048)).astype(BF16)


def get_nc():
    if "nc" not in _CACHE:
        _CACHE["nc"] = _build_nc()
    return _CACHE["nc"]


def make_in_maps(x, Wq, Wk, Wv, Wproj, bproj):
    x = np.asarray(x, dtype=np.float32)
    wq_t = _pack_qk(Wq)
    wk_t = _pack_qk(Wk)
    wv_t = _pack_cn(np.transpose(np.asarray(Wv, np.float32), (1, 0, 2)).reshape(C, C))
    wp_t = _pack_cn(Wproj)
    bp_t = np.asarray(bproj, np.float32).reshape(1, C)
    in_maps = []
    for i in range(NCORES):
        xb = np.ascontiguousarray(
            x[BL * i: BL * (i + 1)].reshape(BL * T, C)).astype(BF16)
        in_maps.append({
            "x": xb, "wq": wq_t, "wk": wk_t, "wv": wv_t, "wp": wp_t, "bp": bp_t,
        })
    return in_maps


def kernel(x, Wq, Wk, Wv, Wproj, bproj):
    from concourse.bass_utils import run_bass_kernel_spmd

    nc = get_nc()
    in_maps = make_in_maps(x, Wq, Wk, Wv, Wproj, bproj)
    trace = bool(int(os.environ.get("KERNEL_TRACE", "0")))
    res = run_bass_kernel_spmd(nc, in_maps, list(range(NCORES)), trace=trace)
    _CACHE["last_result"] = res
    out = np.empty((B, C, HH, WW), np.float32)
    for i in range(NCORES):
        out[BL * i: BL * (i + 1)] = res.results[i]["y"].reshape(BL, C, HH, WW)
    return out


# revision 55
# speedup vs baseline: 1.9336x; 1.0097x over previous
"""Multi-head attention Trainium2 kernel (8 NeuronCores, data-parallel over batch).

Per-core program (2 batches per core), cost-model-optimized, all-bf16:
  x [2048, 512] bf16 --DMA-xbar-transpose--> xts [c,(cc),t] bf16
  QKV projections bf16 (c-tiled accumulation, N=512 per step)
  scores S^T[s,t] = kT (stationary) x qT (moving) bf16 -> exp on ScalarE
  (optional DVE Schraudolph offload) -> P [s,t] bf16
  PV in O[t,d] layout: P^T stationary, V_ext (V|ones) moving bf16, N=65 per
  accumulation step -> rowsum free in col 64; two heads per psum tile in
  disjoint banks (start=True zeroes a whole 2KB bank)
  normalize on GPSIMD (normalize_recip, SBUF->SBUF) -> O_sb bf16
  O_sb -> PE-transpose -> oT [hd,t] bf16 -> proj bf16 + bias add -> y f32
Emission is interleaved in head-pair windows so PE/DVE/Pool work hides
under the ACT (exp) critical path.
"""
import sys
import os
from collections import deque

sys.path.insert(0, "/opt/trn_rl_repo")
import numpy as np
import ml_dtypes

B, C, HH, WW = 16, 512, 32, 32
T = HH * WW               # 1024
NH, HD = 8, 64
BL = 2                    # batches per core
NCORES = 8
EXP_SCALE = 0.125

FP8 = ml_dtypes.float8_e4m3
BF16 = ml_dtypes.bfloat16

# s-tile indices whose exp runs on DVE (Schraudolph bit-trick) instead of ACT
SCHR_ST = set(int(c) for c in os.environ.get("SCHR_ST", "").split(",") if c != "")
# Schraudolph constants for bf16 bits: bits = 2^7 * (log2(P) + 127 - c)
SCHR_A = 128.0 * np.log2(np.e) * EXP_SCALE
SCHR_B = 128.0 * (127.0 - 0.0579) + float(os.environ.get("SCHR_RND", "0.5"))

_CACHE = {}


def _build_nc():
    import concourse.bacc as bacc
    import concourse.mybir as mybir
    import concourse.tile as tile
    from concourse import masks
    from concourse import library_config

    f32 = mybir.dt.float32
    bf16 = mybir.dt.bfloat16
    u16 = mybir.dt.uint16
    Exp = mybir.ActivationFunctionType.Exp
    MUL = mybir.AluOpType.mult

    nc = bacc.Bacc("TRN2", target_bir_lowering=False, debug=False, num_devices=NCORES)
    x = nc.dram_tensor("x", [BL * T, C], bf16, kind="ExternalInput").ap()
    wq = nc.dram_tensor("wq", [128, 2048], bf16, kind="ExternalInput").ap()
    wk = nc.dram_tensor("wk", [128, 2048], bf16, kind="ExternalInput").ap()
    wv = nc.dram_tensor("wv", [128, 2048], bf16, kind="ExternalInput").ap()
    wp = nc.dram_tensor("wp", [128, 2048], bf16, kind="ExternalInput").ap()
    bp = nc.dram_tensor("bp", [1, C], f32, kind="ExternalInput").ap()
    y = nc.dram_tensor("y", [BL * T, C], f32, kind="ExternalOutput").ap()

    with tile.TileContext(nc) as tc:
        with tc.tile_pool(name="const", bufs=1) as cpool, \
             tc.tile_pool(name="xn", bufs=4) as xn_pool, \
             tc.tile_pool(name="xt", bufs=2) as xt_pool, \
             tc.tile_pool(name="qk", bufs=16) as qk_pool, \
             tc.tile_pool(name="vv", bufs=16) as v_pool, \
             tc.tile_pool(name="pp", bufs=34) as p_pool, \
             tc.tile_pool(name="ob", bufs=2) as osb_pool, \
             tc.tile_pool(name="ot", bufs=8) as ot_pool, \
             tc.tile_pool(name="or", bufs=3) as or_pool, \
             tc.tile_pool(name="yy", bufs=5) as y_pool, \
             tc.tile_pool(name="ps", bufs=1, space="PSUM") as ps_pool:

            wq_s = cpool.tile([128, 4, C], bf16, tag="wq")
            wk_s = cpool.tile([128, 4, C], bf16, tag="wk")
            wv_s = cpool.tile([128, 4, C], bf16, tag="wv")
            wp_s = cpool.tile([128, 2048], bf16, tag="wp")
            bias_b = cpool.tile([128, C], f32, tag="bias")
            ident = cpool.tile([128, 128], bf16, tag="ident")
            # DMA order matters: the batch-0 x tiles and wq/wk gate the first
            # scores, so everything else loads after those.
            masks.make_identity(nc, ident[:])
            nc.gpsimd.load_library(library_config.attn)

            def load_wq():
                nc.sync.dma_start(wq_s[:], wq.rearrange("p (a b) -> p a b", a=4))

            def load_wk():
                nc.sync.dma_start(wk_s[:], wk.rearrange("p (a b) -> p a b", a=4))

            def load_consts_rest():
                nc.sync.dma_start(wv_s[:], wv.rearrange("p (a b) -> p a b", a=4))
                nc.sync.dma_start(wp_s[:], wp)
                nc.sync.dma_start(bias_b[:], bp.to_broadcast([128, C]))

            # persistent per-batch state
            xts = {}      # b -> [128, 4, 1024] bf16
            qsb, ksb = {}, {}   # (b, pair) -> [128, 1024] bf16 (qT/kT, 2 heads)
            vsb = {}      # (b, j) -> [128, 8, 65] bf16 (V | ones)
            Pt = {}       # (b, h, st) -> [128, 1024] bf16
            Osb = {}      # b -> [128, 4096] bf16
            oT = {}       # (b, pair) -> [128, 1024] bf16

            def u_tile(name):
                return ps_pool.tile([128, 512], f32, tag="u", bufs=4, name=name)

            def s_tile(name):
                return ps_pool.tile([128, 1024], f32, tag="s", bufs=2, name=name)

            # ---------- prep units ----------
            def prep_units(b, units):
                xts[b] = xt_pool.tile([128, 4, T], bf16, tag="xt", name=f"xts_{b}")
                Osb[b] = osb_pool.tile([128, 8 * C], bf16, tag="osb", name=f"osb_{b}")

                def xt_unit(cc):
                    # batch 1: xbar DMA transpose (no PE/DVE cost, runs in
                    # window slack)
                    def emit():
                        nc.sync.dma_start_transpose(
                            xts[b][:, cc, :],
                            x[b * T:(b + 1) * T, cc * 128:(cc + 1) * 128])
                    return emit

                def xn_unit(tt):
                    # batch 0: PE transpose route — much shorter critical
                    # path at kernel start than 4 serial xbar transposes
                    def emit():
                        xn = xn_pool.tile([128, C], bf16, tag="xn", name=f"xn_{b}_{tt}")
                        nc.sync.dma_start(xn[:], x[b * T + tt * 128: b * T + tt * 128 + 128, :])
                        tr = u_tile(f"tr_{b}_{tt}").bitcast(bf16)
                        for cc in range(4):
                            nc.tensor.transpose(tr[:, cc * 128:(cc + 1) * 128],
                                                xn[:, cc * 128:(cc + 1) * 128], ident[:])
                        nc.vector.tensor_copy(
                            xts[b][:, :, tt * 128:(tt + 1) * 128],
                            tr[:, 0:512].rearrange("p (cc m) -> p cc m", cc=4))
                    return emit

                def qk_unit(wi, p, ch):
                    def emit():
                        wsb = (wq_s, wk_s)[wi]
                        dst = (qsb, ksb)[wi]
                        if (b, p) not in dst:
                            dst[(b, p)] = qk_pool.tile(
                                [128, T], bf16, tag="qk", name=f"qk_{b}_{wi}_{p}")
                        ps_t = u_tile(f"qkp_{b}_{wi}_{p}_{ch}")
                        for cc in range(4):
                            nc.tensor.matmul(
                                ps_t[:],
                                wsb[:, cc, p * 128:(p + 1) * 128],
                                xts[b][:, cc, ch * 512:(ch + 1) * 512],
                                start=(cc == 0), stop=(cc == 3))
                        nc.vector.tensor_copy(
                            dst[(b, p)][:, ch * 512:(ch + 1) * 512], ps_t[:])
                    return emit

                def v_unit(st):
                    def emit():
                        v_ps = u_tile(f"vp_{b}_{st}")
                        for cc in range(4):
                            nc.tensor.matmul(
                                v_ps[:],
                                xts[b][:, cc, st * 128:(st + 1) * 128],
                                wv_s[:, cc, :],
                                start=(cc == 0), stop=(cc == 3))
                        v_t = v_pool.tile([128, NH, HD + 1], bf16, tag="v",
                                          name=f"vs_{b}_{st}")
                        nc.gpsimd.memset(v_t[:, :, HD], 1.0)
                        nc.vector.tensor_copy(
                            v_t[:, :, 0:HD],
                            v_ps[:].rearrange("p (h d) -> p h d", h=NH))
                        vsb[(b, st)] = v_t
                    return emit

                if b == 0:
                    units["xt", b] = [xn_unit(tt) for tt in range(8)]
                else:
                    units["xt", b] = [xt_unit(cc) for cc in range(4)]
                for p in range(4):
                    units["qk", b, p] = [qk_unit(wi, p, ch)
                                         for wi in range(2) for ch in range(2)]
                units["v", b] = [v_unit(st) for st in range(8)]

            # ---------- attention pieces ----------
            def scores_exp(b, h, st, split=False):
                s_ps = s_tile(f"s_{b}_{h}_{st}")
                p, hh = h // 2, h % 2
                p_t = p_pool.tile([128, T], bf16, tag="p", name=f"p_{b}_{h}_{st}")
                for ch in range(2):
                    nc.tensor.matmul(
                        s_ps[:, ch * 512:(ch + 1) * 512],
                        ksb[(b, p)][64 * hh:64 * hh + 64, st * 128:(st + 1) * 128],
                        qsb[(b, p)][64 * hh:64 * hh + 64, ch * 512:(ch + 1) * 512],
                        start=True, stop=True)
                    if split:
                        # half-width exp right after each scores half: lets
                        # ACT start before the second half's operands exist
                        nc.scalar.activation(
                            p_t[:, ch * 512:(ch + 1) * 512],
                            s_ps[:, ch * 512:(ch + 1) * 512], Exp, scale=EXP_SCALE)
                if not split:
                    if st in SCHR_ST:
                        nc.vector.tensor_scalar(
                            p_t[:].bitcast(u16), s_ps[:],
                            scalar1=float(SCHR_A), op0=MUL,
                            scalar2=float(SCHR_B), op1=mybir.AluOpType.add)
                    else:
                        nc.scalar.activation(p_t[:], s_ps[:], Exp, scale=EXP_SCALE)
                Pt[(b, h, st)] = p_t

            def pv_chunk(b, pr, tt):
                # one psum bank (u tile) per head: each accumulation group
                # owns its bank (start=True zeroes a whole 2KB bank).
                ha = 2 * pr
                o_ps = [u_tile(f"o_{b}_{pr}_{tt}_{hi}") for hi in range(2)]
                for j in range(8):
                    for hi in range(2):
                        nc.tensor.matmul(
                            o_ps[hi][:, 0:HD + 1],
                            Pt[(b, ha + hi, j)][:, tt * 128:(tt + 1) * 128],
                            vsb[(b, j)][:, ha + hi, :],
                            start=(j == 0), stop=(j == 7),
                            skip_group_check=True)
                o_raw = or_pool.tile([128, 2, HD + 1], f32, tag="or",
                                     name=f"oraw_{b}_{pr}_{tt}")
                for hi in range(2):
                    nc.vector.tensor_copy(o_raw[:, hi, :], o_ps[hi][:, 0:HD + 1])
                for hi in range(2):
                    nc.gpsimd.normalize_recip(
                        Osb[b][:, tt * 512 + (ha + hi) * HD:
                               tt * 512 + (ha + hi + 1) * HD],
                        o_raw[:, hi, 0:HD],
                        o_raw[:, hi, HD:HD + 1])

            def ot_unit(b, pr):
                otp = u_tile(f"otp_{b}_{pr}").bitcast(bf16)
                for tt in range(8):
                    nc.tensor.transpose(
                        otp[:, tt * 128:(tt + 1) * 128],
                        Osb[b][:, tt * 512 + pr * 128: tt * 512 + (pr + 1) * 128],
                        ident[:])
                o_t = ot_pool.tile([128, T], bf16, tag="ot", name=f"oT_{b}_{pr}")
                nc.vector.tensor_copy(o_t[:], otp[:])
                oT[(b, pr)] = o_t

            y01 = {}

            def proj_units(b):
                units = []

                def proj_unit(tt):
                    def emit():
                        y_ps = u_tile(f"yp_{b}_{tt}")
                        for p in range(4):
                            nc.tensor.matmul(y_ps[:],
                                             oT[(b, p)][:, tt * 128:(tt + 1) * 128],
                                             wp_s[:, p * 512:(p + 1) * 512],
                                             start=(p == 0), stop=(p == 3))
                        y_sb = y_pool.tile([128, C], f32, tag="y", name=f"ys_{b}_{tt}")
                        nc.vector.tensor_add(y_sb[:], y_ps[:], bias_b[:])
                        nc.sync.dma_start(y[b * T + tt * 128: b * T + tt * 128 + 128, :],
                                          y_sb[:])
                    return emit

                for tt in range(8):
                    units.append(proj_unit(tt))
                return units

            def proj01_units(b):
                # first half of the output projection (pairs 0/1 + bias),
                # runnable as soon as oT[(b,0..1)] exist — fills window slack
                units = []

                def unit(tt):
                    def emit():
                        y_ps = u_tile(f"yh_{b}_{tt}")
                        for p in range(2):
                            nc.tensor.matmul(y_ps[:],
                                             oT[(b, p)][:, tt * 128:(tt + 1) * 128],
                                             wp_s[:, p * 512:(p + 1) * 512],
                                             start=(p == 0), stop=(p == 1))
                        yh = y_pool.tile([128, C], bf16, tag="y01", bufs=8,
                                         name=f"yh_{b}_{tt}")
                        nc.vector.tensor_add(yh[:], y_ps[:], bias_b[:])
                        y01[(b, tt)] = yh
                    return emit

                for tt in range(8):
                    units.append(unit(tt))
                return units

            def proj23_units(b):
                units = []

                def unit(tt):
                    def emit():
                        y_ps = u_tile(f"yt_{b}_{tt}")
                        for p in range(2, 4):
                            nc.tensor.matmul(y_ps[:],
                                             oT[(b, p)][:, tt * 128:(tt + 1) * 128],
                                             wp_s[:, p * 512:(p + 1) * 512],
                                             start=(p == 2), stop=(p == 3))
                        y_sb = y_pool.tile([128, C], f32, tag="y", name=f"ys_{b}_{tt}")
                        nc.vector.tensor_tensor(y_sb[:], y_ps[:], y01[(b, tt)][:],
                                                op=mybir.AluOpType.add)
                        nc.sync.dma_start(y[b * T + tt * 128: b * T + tt * 128 + 128, :],
                                          y_sb[:])
                    return emit

                for tt in range(8):
                    units.append(unit(tt))
                return units

            # ---------- emission: 8 head-pair windows ----------
            U = {}
            prep_units(0, U)
            prep_units(1, U)
            # head: x transposes + pair-0 q/k of batch 0. ch0 of q/k only
            # needs the first 4 t-tiles transposed, so interleave.
            qk00 = U["qk", 0, 0]   # order: (q,ch0), (q,ch1), (k,ch0), (k,ch1)
            for u in U["xt", 0][0:4]:
                u()
            load_wq()
            load_wk()
            qk00[0]()
            for u in U["xt", 0][4:6]:
                u()
            qk00[2]()
            for u in U["xt", 0][6:8]:
                u()
            qk00[1]()
            qk00[3]()
            load_consts_rest()
            # per-window filler schedule (deadline: qk(b,p) before window of
            # pair (b,p); v(b) before the first pv_chunk of batch b's pairs)
            wfill = [
                U["qk", 0, 1] + U["v", 0],          # W0 (no pv in W0)
                U["qk", 0, 2] + U["xt", 1],         # W1
                U["qk", 0, 3] + U["qk", 1, 0],      # W2
                U["qk", 1, 1] + U["qk", 1, 2],      # W3
                U["v", 1],                          # W4
                U["qk", 1, 3],                      # W5
                proj_units(0),                      # W6
                proj01_units(1),                    # W7
            ]
            pairs = [(b, pr) for b in range(2) for pr in range(4)]
            for wi_, (b, pr) in enumerate(pairs):
                # oT for the pair whose PV finished at the end of last window
                if wi_ >= 2:
                    ot_unit(*pairs[wi_ - 2])
                fill = deque(wfill[wi_])
                for st in range(8):
                    for hi in range(2):
                        scores_exp(b, 2 * pr + hi, st,
                                   split=(wi_ == 0 and st < 2))
                    if wi_ >= 1:
                        pb, ppr = pairs[wi_ - 1]
                        pv_chunk(pb, ppr, st)
                    # pace fillers lightly at window start (exp supply is
                    # tightest there), catch up later
                    k = (len(fill) + 7 - st) // (8 - st)
                    k = min(k, 1 if st < 2 else 4)
                    for _ in range(k):
                        if fill:
                            fill.popleft()()
                while fill:
                    fill.popleft()()
            # tail: fully pipelined per tt — pv -> transpose -> oT-slice copy
            # -> proj23 -> y add -> store (s pool is free for otp3 now)
            ot_unit(1, 2)
            otp3 = s_tile("otp_1_3").bitcast(bf16)
            o_t3 = ot_pool.tile([128, T], bf16, tag="ot", name="oT_1_3")
            oT[(1, 3)] = o_t3
            p23 = proj23_units(1)

            def tail_unit(tt):
                # transpose + oT-slice copy only; proj23 emitted after all
                # copies so the DVE queue never blocks the next tt's copy
                nc.tensor.transpose(
                    otp3[:, tt * 128:(tt + 1) * 128],
                    Osb[1][:, tt * 512 + 3 * 128: tt * 512 + 4 * 128],
                    ident[:])
                nc.vector.tensor_copy(o_t3[:, tt * 128:(tt + 1) * 128],
                                      otp3[:, tt * 128:(tt + 1) * 128])

            for tt in range(8):
                pv_chunk(1, 3, tt)
                if tt >= 3:
                    tail_unit(tt - 3)
            for tt in range(5, 8):
                tail_unit(tt)
            for tt in range(8):
                p23[tt]()

    nc.compile()
    return nc


def _pack_qk(w):
    # [NH, C, HD] -> [c, h*HD+d] -> tiled [c_local, cc, d] -> [128, 2048] bf16
    wn = np.transpose(np.asarray(w, np.float32), (1, 0, 2)).reshape(C, C)
    return np.ascontiguousarray(
        wn.reshape(4, 128, C).transpose(1, 0, 2).reshape(128, 2048)).astype(BF16)


def _pack_cn(wn):
    # [C, N] natural -> tiled [c_local, cc, n] -> [128, 2048] bf16
    return np.ascontiguousarray(
        np.asarray(wn, np.float32).reshape(4, 128, C)
        .transpose(1, 0, 2).reshape(128, 2048)).astype(BF16)


def get_nc():
    if "nc" not in _CACHE:
        _CACHE["nc"] = _build_nc()
    return _CACHE["nc"]


def make_in_maps(x, Wq, Wk, Wv, Wproj, bproj):
    x = np.asarray(x, dtype=np.float32)
    wq_t = _pack_qk(Wq)
    wk_t = _pack_qk(Wk)
    wv_t = _pack_cn(np.transpose(np.asarray(Wv, np.float32), (1, 0, 2)).reshape(C, C))
    wp_t = _pack_cn(Wproj)
    bp_t = np.asarray(bproj, np.float32).reshape(1, C)
    in_maps = []
    for i in range(NCORES):
        xb = np.ascontiguousarray(
            x[BL * i: BL * (i + 1)].reshape(BL * T, C)).astype(BF16)
        in_maps.append({
            "x": xb, "wq": wq_t, "wk": wk_t, "wv": wv_t, "wp": wp_t, "bp": bp_t,
        })
    return in_maps


def kernel(x, Wq, Wk, Wv, Wproj, bproj):
    from concourse.bass_utils import run_bass_kernel_spmd

    nc = get_nc()
    in_maps = make_in_maps(x, Wq, Wk, Wv, Wproj, bproj)
    trace = bool(int(os.environ.get("KERNEL_TRACE", "0")))
    res = run_bass_kernel_spmd(nc, in_maps, list(range(NCORES)), trace=trace)
    _CACHE["last_result"] = res
    out = np.empty((B, C, HH, WW), np.float32)
    for i in range(NCORES):
        out[BL * i: BL * (i + 1)] = res.results[i]["y"].reshape(BL, C, HH, WW)
    return out
